# revision 4
# baseline (speedup 1.0000x reference)
"""Trainium2 Bass kernel for nn_ConditionalSmilesRnn (3-layer LSTM, B=256, H=1024, V=47).

Strategy: 8-way feature/model-parallel sharding. Each core owns a 128-row slice
of the hidden dim (and the matching 4x128 rows of every 4096-row gate matrix),
keeps all its weight slices resident in SBUF as float32r (TF32-like PE mode,
full PE rate), and runs the full batch B=256 as the matmul moving operand.
The (layer, time) recurrence runs as a wavefront over diagonals d = t + layer;
each layer-step's hidden-state slice is AllGather'd across the 8 cores, and the
gathers are staggered so they hide under the PE work of neighboring diagonals.

Layer-0's input path (embedding + properties + biases) is algebraically
precomputed on the host: gx0[b,t,:] = (E @ W_ih0[:, :H].T)[tokens[b,t]]
+ props[b] @ W_ih0[:, H:].T + b_ih0 + b_hh0, then streamed per step.

The decoder (V=47, padded to 128) is computed redundantly on every core
(avoids SPMD control-flow divergence); the host takes core 0's copy.
"""
import numpy as np

B = 256
H = 1024
V = 47
P = 3
L = 3
NCORES = 8
BEGIN_IDX = 1
KT = H // 128  # 8 k-tiles of the contraction dim
NG = 4         # gate groups: i, f, g, o
DEC_M = 128    # vocab padded to full partition tile for the decoder matmul

_BUILD_CACHE = {}


def _build(steps):
    import concourse.bacc as bacc
    import concourse.tile as tile
    import concourse.mybir as mybir

    F32 = mybir.dt.float32
    F32R = mybir.dt.float32r
    AF = mybir.ActivationFunctionType

    nc = bacc.Bacc("TRN2", target_bir_lowering=False, debug=False,
                   num_devices=NCORES)

    wts = nc.dram_tensor("wts", [128, 5, NG, KT, 128], F32, kind="ExternalInput")
    g0 = nc.dram_tensor("g0", [steps, 128, NG, B], F32, kind="ExternalInput")
    h0_in = nc.dram_tensor("h0_in", [L, 128, KT, B], F32, kind="ExternalInput")
    c0_in = nc.dram_tensor("c0_in", [L, 128, B], F32, kind="ExternalInput")
    bias_in = nc.dram_tensor("bias_in", [128, 2, NG], F32, kind="ExternalInput")
    wdec_in = nc.dram_tensor("wdec_in", [128, KT, DEC_M], F32, kind="ExternalInput")
    bdec_in = nc.dram_tensor("bdec_in", [DEC_M, 1], F32, kind="ExternalInput")

    logits_out = nc.dram_tensor("logits_out", [steps, V, B], F32, kind="ExternalOutput")
    hT_out = nc.dram_tensor("hT_out", [L, 128, KT, B], F32, kind="ExternalOutput")
    cT_out = nc.dram_tensor("cT_out", [L, 128, B], F32, kind="ExternalOutput")

    g0_ap = g0.ap()
    h0_ap = h0_in.ap()
    c0_ap = c0_in.ap()
    lo_ap = logits_out.ap()
    hT_ap = hT_out.ap()
    cT_ap = cT_out.ap()

    ndiag = steps + 3
    WHH0, WIH1, WHH1, WIH2, WHH2 = range(5)

    with tile.TileContext(nc) as tc:
        with (
            tc.tile_pool(name="const", bufs=1) as constp,
            tc.tile_pool(name="state", bufs=1) as statep,
            tc.tile_pool(name="gx", bufs=3) as gxp,
            tc.tile_pool(name="gates", bufs=3) as gatesp,
            tc.tile_pool(name="hsl", bufs=2) as hslp,
            tc.tile_pool(name="dec", bufs=2) as decp,
            tc.tile_pool(name="ps", bufs=1, space="PSUM") as psp,
            tc.tile_pool(name="psdec", bufs=1, space="PSUM") as psdecp,
            tc.tile_pool(name="dram", bufs=2, space="DRAM") as dramp,
        ):
            wts_sb = constp.tile([128, 5, NG, KT, 128], F32R)
            nc.gpsimd.dma_start(wts_sb[:], wts.ap())
            wdec_sb = constp.tile([128, KT, DEC_M], F32R)
            nc.gpsimd.dma_start(wdec_sb[:], wdec_in.ap())
            bias_sb = constp.tile([128, 2, NG], F32)
            nc.sync.dma_start(bias_sb[:], bias_in.ap())
            bdec_sb = constp.tile([DEC_M, 1], F32)
            nc.sync.dma_start(bdec_sb[:], bdec_in.ap())

            h_sb = [statep.tile([128, KT, B], F32R, tag=f"h{l}", name=f"h_sb{l}")
                    for l in range(L)]
            c_sb = [statep.tile([128, B], F32, tag=f"c{l}", name=f"c_sb{l}")
                    for l in range(L)]
            for l in range(L):
                nc.gpsimd.dma_start(h_sb[l][:], h0_ap[l])
                nc.sync.dma_start(c_sb[l][:], c0_ap[l])

            def gemm_layer(l, ps):
                if l == 0:
                    for g in range(NG):
                        for k in range(KT):
                            nc.tensor.matmul(
                                ps[:, g * B:(g + 1) * B],
                                wts_sb[:, WHH0, g, k, :], h_sb[0][:, k, :],
                                start=(k == 0), stop=(k == KT - 1))
                else:
                    ihm = WIH1 if l == 1 else WIH2
                    hhm = WHH1 if l == 1 else WHH2
                    for g in range(NG):
                        for k in range(KT):
                            nc.tensor.matmul(
                                ps[:, g * B:(g + 1) * B],
                                wts_sb[:, hhm, g, k, :], h_sb[l][:, k, :],
                                start=(k == 0), stop=False)
                        for k in range(KT):
                            nc.tensor.matmul(
                                ps[:, g * B:(g + 1) * B],
                                wts_sb[:, ihm, g, k, :], h_sb[l - 1][:, k, :],
                                start=False, stop=(k == KT - 1))

            def cell(l, ps, gx=None):
                acts = []
                for g in range(NG):
                    pslice = ps[:, g * B:(g + 1) * B]
                    a = gatesp.tile([128, B], F32, tag=f"a{g}")
                    func = AF.Tanh if g == 2 else AF.Sigmoid
                    if l == 0:
                        ssum = gatesp.tile([128, B], F32, tag=f"s{g}")
                        nc.vector.tensor_add(ssum[:], pslice, gx[:, g, :])
                        nc.scalar.activation(a[:], ssum[:], func)
                    else:
                        nc.scalar.activation(a[:], pslice, func,
                                             bias=bias_sb[:, l - 1, g:g + 1])
                    acts.append(a)
                i_, f_, g_, o_ = acts
                ig = gatesp.tile([128, B], F32, tag="ig")
                nc.vector.tensor_mul(ig[:], i_[:], g_[:])
                fc = gatesp.tile([128, B], F32, tag="fc")
                nc.vector.tensor_mul(fc[:], f_[:], c_sb[l][:])
                nc.vector.tensor_add(c_sb[l][:], ig[:], fc[:])
                tch = gatesp.tile([128, B], F32, tag="tc")
                nc.scalar.activation(tch[:], c_sb[l][:], AF.Tanh)
                hsl = hslp.tile([128, B], F32R, tag=f"hsl{l}")
                nc.vector.tensor_mul(hsl[:], o_[:], tch[:])
                return hsl

            def allgather(l, hsl):
                agin = dramp.tile([128, B], F32R, tag=f"agin{l}")
                agout = dramp.tile([NCORES * 128, B], F32R, tag=f"agout{l}")
                nc.sync.dma_start(agin[:], hsl[:])
                nc.gpsimd.collective_compute(
                    "AllGather", mybir.AluOpType.bypass,
                    replica_groups=[list(range(NCORES))],
                    ins=[agin.opt()], outs=[agout.opt()])
                nc.sync.dma_start(
                    h_sb[l][:],
                    agout[:].rearrange("(k p) b -> p k b", p=128))

            for d in range(ndiag):
                # Phase 1: all readers of the previous-diagonal h state.
                # Layer l at diagonal d computes t = d - l; its GEMMs (and the
                # decoder) must run before any allgather of this diagonal
                # overwrites h_sb with this diagonal's h.
                ps_l = [None] * L
                gx = None
                if d <= steps - 1:
                    gx = gxp.tile([128, NG, B], F32, tag="gx")
                    nc.sync.dma_start(gx[:], g0_ap[d])
                    ps_l[0] = psp.tile([128, NG * B], F32, tag="ps0", name="ps0")
                    gemm_layer(0, ps_l[0])
                if 1 <= d <= steps:
                    ps_l[1] = psp.tile([128, NG * B], F32, tag="ps1", name="ps1")
                    gemm_layer(1, ps_l[1])
                if 2 <= d <= steps + 1:
                    ps_l[2] = psp.tile([128, NG * B], F32, tag="ps2", name="ps2")
                    gemm_layer(2, ps_l[2])
                td = d - 3
                if 0 <= td <= steps - 1:
                    psd = psdecp.tile([DEC_M, B], F32, tag="psd")
                    for k in range(KT):
                        nc.tensor.matmul(psd[:], wdec_sb[:, k, :],
                                         h_sb[2][:, k, :],
                                         start=(k == 0), stop=(k == KT - 1))
                    dec = decp.tile([V, B], F32, tag="dec")
                    nc.scalar.activation(dec[:], psd[:V, :], AF.Identity,
                                         bias=bdec_sb[:V, :])
                    nc.sync.dma_start(lo_ap[td], dec[:])
                # Phase 2: cells + staggered allgathers (h_sb writers).
                if d <= steps - 1:
                    allgather(0, cell(0, ps_l[0], gx))
                if 1 <= d <= steps:
                    allgather(1, cell(1, ps_l[1]))
                if 2 <= d <= steps + 1:
                    allgather(2, cell(2, ps_l[2]))

            for l in range(L):
                nc.sync.dma_start(hT_ap[l], h_sb[l][:].bitcast(F32))
                nc.sync.dma_start(cT_ap[l], c_sb[l][:])
    nc.compile()
    return nc


def _get_nc(steps):
    if steps not in _BUILD_CACHE:
        _BUILD_CACHE[steps] = _build(steps)
    return _BUILD_CACHE[steps]


def kernel(x, properties, h0, c0, E, W_ih0, W_hh0, b_ih0, b_hh0,
           W_ih_r, W_hh_r, b_ih_r, b_hh_r, W_dec, b_dec, seq_len):
    from concourse.bass_utils import run_bass_kernel_spmd

    x = np.asarray(x)
    properties = np.asarray(properties, dtype=np.float32)
    h0 = np.asarray(h0, dtype=np.float32)
    c0 = np.asarray(c0, dtype=np.float32)
    E = np.asarray(E, dtype=np.float32)
    W_ih0 = np.asarray(W_ih0, dtype=np.float32)
    W_hh0 = np.asarray(W_hh0, dtype=np.float32)
    b_ih0 = np.asarray(b_ih0, dtype=np.float32)
    b_hh0 = np.asarray(b_hh0, dtype=np.float32)
    W_ih_r = np.asarray(W_ih_r, dtype=np.float32)
    W_hh_r = np.asarray(W_hh_r, dtype=np.float32)
    b_ih_r = np.asarray(b_ih_r, dtype=np.float32)
    b_hh_r = np.asarray(b_hh_r, dtype=np.float32)
    W_dec = np.asarray(W_dec, dtype=np.float32)
    b_dec = np.asarray(b_dec, dtype=np.float32)
    steps = int(seq_len) - 1

    # host-side layer-0 input path: gx0[b,t,:] = EW[tokens] + props/bias terms
    tokens = np.concatenate(
        [np.full((B, 1), BEGIN_IDX, dtype=np.int64),
         x[:, 1:steps].astype(np.int64)], axis=1)          # [B, steps]
    EW = E @ W_ih0[:, :H].T                                # [V, 4H]
    PP = properties @ W_ih0[:, H:].T + b_ih0 + b_hh0       # [B, 4H]
    gx0 = EW[tokens] + PP[:, None, :]                      # [B, steps, 4H] f32

    mats = [W_hh0, W_ih_r[0], W_hh_r[0], W_ih_r[1], W_hh_r[1]]
    bb = (b_ih_r + b_hh_r).reshape(2, NG, NCORES, 128)     # [layer, g, j, p]
    Wd = np.zeros((DEC_M, H), np.float32)
    Wd[:V] = W_dec
    wdec_host = np.ascontiguousarray(
        Wd.reshape(DEC_M, KT, 128).transpose(2, 1, 0))     # [kp, k, m]
    bdec_host = np.zeros((DEC_M, 1), np.float32)
    bdec_host[:V, 0] = b_dec
    gx0_r = gx0.reshape(B, steps, NG, NCORES, 128)         # [b, t, g, j, m]
    h0_r = h0.reshape(L, B, KT, 128)                       # [l, b, k, p]
    h0_host = np.ascontiguousarray(h0_r.transpose(0, 3, 2, 1))  # [l, p, k, b]
    c0_r = c0.reshape(L, B, NCORES, 128)

    in_maps = []
    for j in range(NCORES):
        wts_host = np.empty((128, 5, NG, KT, 128), np.float32)
        for mi, W in enumerate(mats):
            Wr = W.reshape(NG, NCORES, 128, KT, 128)       # [g, j, m, k, kp]
            wts_host[:, mi] = Wr[:, j].transpose(3, 0, 2, 1)  # [kp, g, k, m]
        in_maps.append({
            "wts": wts_host,
            "g0": np.ascontiguousarray(gx0_r[:, :, :, j, :].transpose(1, 3, 2, 0)),
            "h0_in": h0_host,
            "c0_in": np.ascontiguousarray(c0_r[:, :, j, :].transpose(0, 2, 1)),
            "bias_in": np.ascontiguousarray(bb[:, :, j, :].transpose(2, 0, 1)),
            "wdec_in": wdec_host,
            "bdec_in": bdec_host,
        })

    nc = _get_nc(steps)
    res = run_bass_kernel_spmd(nc, in_maps, core_ids=list(range(NCORES)))

    r0 = res.results[0]
    logits = np.ascontiguousarray(
        r0["logits_out"].transpose(2, 0, 1))               # [B, steps, V]
    hT = np.ascontiguousarray(
        r0["hT_out"].transpose(0, 3, 2, 1)).reshape(L, B, H)
    cT = np.empty((L, B, H), np.float32)
    for j in range(NCORES):
        cj = res.results[j]["cT_out"]                      # [L, 128, B]
        cT[:, :, j * 128:(j + 1) * 128] = cj.transpose(0, 2, 1)
    return logits, hT, cT


# revision 7
# speedup vs baseline: 1.8882x; 1.8882x over previous
"""Trainium2 Bass kernel for nn_ConditionalSmilesRnn (3-layer LSTM, B=256, H=1024, V=47).

Strategy: 8-way feature/model-parallel sharding. Each core owns a 128-row slice
of the hidden dim (and the matching 4x128 rows of every 4096-row gate matrix),
keeps all its weight slices resident in SBUF as float32r (TF32-like PE mode,
full PE rate), and runs the full batch B=256 as the matmul moving operand.
The (layer, time) recurrence runs as a wavefront over diagonals d = t + layer;
each layer-step's hidden-state slice is AllGather'd across the 8 cores, and the
gathers are staggered so they hide under the PE work of neighboring diagonals.

Layer-0's input path (embedding + properties + biases) is algebraically
precomputed on the host: gx0[b,t,:] = (E @ W_ih0[:, :H].T)[tokens[b,t]]
+ props[b] @ W_ih0[:, H:].T + b_ih0 + b_hh0, then streamed per step.

The decoder (V=47, padded to 128) is computed redundantly on every core
(avoids SPMD control-flow divergence); the host takes core 0's copy.
"""
import numpy as np

B = 256
H = 1024
V = 47
P = 3
L = 3
NCORES = 8
BEGIN_IDX = 1
KT = H // 128  # 8 k-tiles of the contraction dim
NG = 4         # gate groups: i, f, g, o
DEC_M = 128    # vocab padded to full partition tile for the decoder matmul

_BUILD_CACHE = {}


def _build(steps, nocc=False):
    import concourse.bacc as bacc
    import concourse.tile as tile
    import concourse.mybir as mybir

    F32 = mybir.dt.float32
    F32R = mybir.dt.float32r
    AF = mybir.ActivationFunctionType

    nc = bacc.Bacc("TRN2", target_bir_lowering=False, debug=False,
                   num_devices=NCORES)

    wts = nc.dram_tensor("wts", [128, 5, NG, KT, 128], F32, kind="ExternalInput")
    g0 = nc.dram_tensor("g0", [steps, 128, NG, B], F32, kind="ExternalInput")
    h0_in = nc.dram_tensor("h0_in", [L, 128, KT, B], F32, kind="ExternalInput")
    c0_in = nc.dram_tensor("c0_in", [L, 128, B], F32, kind="ExternalInput")
    bias_in = nc.dram_tensor("bias_in", [128, 2, NG], F32, kind="ExternalInput")
    wdec_in = nc.dram_tensor("wdec_in", [128, KT, DEC_M], F32, kind="ExternalInput")
    bdec_in = nc.dram_tensor("bdec_in", [DEC_M, 1], F32, kind="ExternalInput")

    logits_out = nc.dram_tensor("logits_out", [steps, V, B], F32, kind="ExternalOutput")
    hT_out = nc.dram_tensor("hT_out", [L, 128, KT, B], F32, kind="ExternalOutput")
    cT_out = nc.dram_tensor("cT_out", [L, 128, B], F32, kind="ExternalOutput")

    g0_ap = g0.ap()
    h0_ap = h0_in.ap()
    c0_ap = c0_in.ap()
    lo_ap = logits_out.ap()
    hT_ap = hT_out.ap()
    cT_ap = cT_out.ap()

    ndiag = steps + 3
    WHH0, WIH1, WHH1, WIH2, WHH2 = range(5)

    with tile.TileContext(nc) as tc:
        with (
            tc.tile_pool(name="const", bufs=1) as constp,
            tc.tile_pool(name="state", bufs=1) as statep,
            tc.tile_pool(name="gx", bufs=3) as gxp,
            tc.tile_pool(name="gates", bufs=3) as gatesp,
            tc.tile_pool(name="hsl", bufs=2) as hslp,
            tc.tile_pool(name="dec", bufs=2) as decp,
            tc.tile_pool(name="ps", bufs=1, space="PSUM") as psp,
            tc.tile_pool(name="psdec", bufs=1, space="PSUM") as psdecp,
            tc.tile_pool(name="dram", bufs=2, space="DRAM") as dramp,
        ):
            wts_sb = constp.tile([128, 5, NG, KT, 128], F32R)
            nc.gpsimd.dma_start(wts_sb[:], wts.ap())
            wdec_sb = constp.tile([128, KT, DEC_M], F32R)
            nc.gpsimd.dma_start(wdec_sb[:], wdec_in.ap())
            bias_sb = constp.tile([128, 2, NG], F32)
            nc.sync.dma_start(bias_sb[:], bias_in.ap())
            bdec_sb = constp.tile([DEC_M, 1], F32)
            nc.sync.dma_start(bdec_sb[:], bdec_in.ap())

            h_sb = [statep.tile([128, KT, B], F32R, tag=f"h{l}", name=f"h_sb{l}")
                    for l in range(L)]
            c_sb = [statep.tile([128, B], F32, tag=f"c{l}", name=f"c_sb{l}")
                    for l in range(L)]
            for l in range(L):
                nc.gpsimd.dma_start(h_sb[l][:], h0_ap[l])
                nc.sync.dma_start(c_sb[l][:], c0_ap[l])

            def gemm_layer(l, ps):
                if l == 0:
                    for g in range(NG):
                        for k in range(KT):
                            nc.tensor.matmul(
                                ps[:, g * B:(g + 1) * B],
                                wts_sb[:, WHH0, g, k, :], h_sb[0][:, k, :],
                                start=(k == 0), stop=(k == KT - 1))
                else:
                    ihm = WIH1 if l == 1 else WIH2
                    hhm = WHH1 if l == 1 else WHH2
                    for g in range(NG):
                        for k in range(KT):
                            nc.tensor.matmul(
                                ps[:, g * B:(g + 1) * B],
                                wts_sb[:, hhm, g, k, :], h_sb[l][:, k, :],
                                start=(k == 0), stop=False)
                        for k in range(KT):
                            nc.tensor.matmul(
                                ps[:, g * B:(g + 1) * B],
                                wts_sb[:, ihm, g, k, :], h_sb[l - 1][:, k, :],
                                start=False, stop=(k == KT - 1))

            def cell(l, ps, gx=None):
                acts = []
                for g in range(NG):
                    pslice = ps[:, g * B:(g + 1) * B]
                    a = gatesp.tile([128, B], F32, tag=f"a{g}")
                    func = AF.Tanh if g == 2 else AF.Sigmoid
                    if l == 0:
                        ssum = gatesp.tile([128, B], F32, tag=f"s{g}")
                        nc.vector.tensor_add(ssum[:], pslice, gx[:, g, :])
                        nc.scalar.activation(a[:], ssum[:], func)
                    else:
                        nc.scalar.activation(a[:], pslice, func,
                                             bias=bias_sb[:, l - 1, g:g + 1])
                    acts.append(a)
                i_, f_, g_, o_ = acts
                ig = gatesp.tile([128, B], F32, tag="ig")
                nc.vector.tensor_mul(ig[:], i_[:], g_[:])
                fc = gatesp.tile([128, B], F32, tag="fc")
                nc.vector.tensor_mul(fc[:], f_[:], c_sb[l][:])
                nc.vector.tensor_add(c_sb[l][:], ig[:], fc[:])
                tch = gatesp.tile([128, B], F32, tag="tc")
                nc.scalar.activation(tch[:], c_sb[l][:], AF.Tanh)
                hsl = hslp.tile([128, B], F32R, tag=f"hsl{l}")
                nc.vector.tensor_mul(hsl[:], o_[:], tch[:])
                return hsl

            def allgather(l, hsl):
                agin = dramp.tile([128, B], F32R, tag=f"agin{l}")
                # Shared address space lets the AG write peer buffers
                # directly (the Local-output path is much slower)
                agout = dramp.tile([NCORES * 128, B], F32R, tag=f"agout{l}",
                                   addr_space="Shared")
                nc.sync.dma_start(agin[:], hsl[:])
                if nocc:
                    nc.sync.dma_start(agout[0:128, :], agin[:])
                else:
                    nc.gpsimd.collective_compute(
                        "AllGather", mybir.AluOpType.bypass,
                        replica_groups=[list(range(NCORES))],
                        ins=[agin.opt()], outs=[agout.opt()])
                nc.sync.dma_start(
                    h_sb[l][:],
                    agout[:].rearrange("(k p) b -> p k b", p=128))

            for d in range(ndiag):
                # Phase 1: all readers of the previous-diagonal h state.
                # Layer l at diagonal d computes t = d - l; its GEMMs (and the
                # decoder) must run before any allgather of this diagonal
                # overwrites h_sb with this diagonal's h.
                ps_l = [None] * L
                gx = None
                if d <= steps - 1:
                    gx = gxp.tile([128, NG, B], F32, tag="gx")
                    nc.sync.dma_start(gx[:], g0_ap[d])
                    ps_l[0] = psp.tile([128, NG * B], F32, tag="ps0", name="ps0")
                    gemm_layer(0, ps_l[0])
                if 1 <= d <= steps:
                    ps_l[1] = psp.tile([128, NG * B], F32, tag="ps1", name="ps1")
                    gemm_layer(1, ps_l[1])
                if 2 <= d <= steps + 1:
                    ps_l[2] = psp.tile([128, NG * B], F32, tag="ps2", name="ps2")
                    gemm_layer(2, ps_l[2])
                td = d - 3
                if 0 <= td <= steps - 1:
                    psd = psdecp.tile([DEC_M, B], F32, tag="psd")
                    for k in range(KT):
                        nc.tensor.matmul(psd[:], wdec_sb[:, k, :],
                                         h_sb[2][:, k, :],
                                         start=(k == 0), stop=(k == KT - 1))
                    dec = decp.tile([V, B], F32, tag="dec")
                    nc.scalar.activation(dec[:], psd[:V, :], AF.Identity,
                                         bias=bdec_sb[:V, :])
                    nc.sync.dma_start(lo_ap[td], dec[:])
                # Phase 2: cells + staggered allgathers (h_sb writers).
                if d <= steps - 1:
                    allgather(0, cell(0, ps_l[0], gx))
                if 1 <= d <= steps:
                    allgather(1, cell(1, ps_l[1]))
                if 2 <= d <= steps + 1:
                    allgather(2, cell(2, ps_l[2]))

            for l in range(L):
                nc.sync.dma_start(hT_ap[l], h_sb[l][:].bitcast(F32))
                nc.sync.dma_start(cT_ap[l], c_sb[l][:])
    nc.compile()
    return nc


def _get_nc(steps):
    if steps not in _BUILD_CACHE:
        _BUILD_CACHE[steps] = _build(steps)
    return _BUILD_CACHE[steps]


def kernel(x, properties, h0, c0, E, W_ih0, W_hh0, b_ih0, b_hh0,
           W_ih_r, W_hh_r, b_ih_r, b_hh_r, W_dec, b_dec, seq_len):
    from concourse.bass_utils import run_bass_kernel_spmd

    x = np.asarray(x)
    properties = np.asarray(properties, dtype=np.float32)
    h0 = np.asarray(h0, dtype=np.float32)
    c0 = np.asarray(c0, dtype=np.float32)
    E = np.asarray(E, dtype=np.float32)
    W_ih0 = np.asarray(W_ih0, dtype=np.float32)
    W_hh0 = np.asarray(W_hh0, dtype=np.float32)
    b_ih0 = np.asarray(b_ih0, dtype=np.float32)
    b_hh0 = np.asarray(b_hh0, dtype=np.float32)
    W_ih_r = np.asarray(W_ih_r, dtype=np.float32)
    W_hh_r = np.asarray(W_hh_r, dtype=np.float32)
    b_ih_r = np.asarray(b_ih_r, dtype=np.float32)
    b_hh_r = np.asarray(b_hh_r, dtype=np.float32)
    W_dec = np.asarray(W_dec, dtype=np.float32)
    b_dec = np.asarray(b_dec, dtype=np.float32)
    steps = int(seq_len) - 1

    # host-side layer-0 input path: gx0[b,t,:] = EW[tokens] + props/bias terms
    tokens = np.concatenate(
        [np.full((B, 1), BEGIN_IDX, dtype=np.int64),
         x[:, 1:steps].astype(np.int64)], axis=1)          # [B, steps]
    EW = E @ W_ih0[:, :H].T                                # [V, 4H]
    PP = properties @ W_ih0[:, H:].T + b_ih0 + b_hh0       # [B, 4H]
    gx0 = EW[tokens] + PP[:, None, :]                      # [B, steps, 4H] f32

    mats = [W_hh0, W_ih_r[0], W_hh_r[0], W_ih_r[1], W_hh_r[1]]
    bb = (b_ih_r + b_hh_r).reshape(2, NG, NCORES, 128)     # [layer, g, j, p]
    Wd = np.zeros((DEC_M, H), np.float32)
    Wd[:V] = W_dec
    wdec_host = np.ascontiguousarray(
        Wd.reshape(DEC_M, KT, 128).transpose(2, 1, 0))     # [kp, k, m]
    bdec_host = np.zeros((DEC_M, 1), np.float32)
    bdec_host[:V, 0] = b_dec
    gx0_r = gx0.reshape(B, steps, NG, NCORES, 128)         # [b, t, g, j, m]
    h0_r = h0.reshape(L, B, KT, 128)                       # [l, b, k, p]
    h0_host = np.ascontiguousarray(h0_r.transpose(0, 3, 2, 1))  # [l, p, k, b]
    c0_r = c0.reshape(L, B, NCORES, 128)

    in_maps = []
    for j in range(NCORES):
        wts_host = np.empty((128, 5, NG, KT, 128), np.float32)
        for mi, W in enumerate(mats):
            Wr = W.reshape(NG, NCORES, 128, KT, 128)       # [g, j, m, k, kp]
            wts_host[:, mi] = Wr[:, j].transpose(3, 0, 2, 1)  # [kp, g, k, m]
        in_maps.append({
            "wts": wts_host,
            "g0": np.ascontiguousarray(gx0_r[:, :, :, j, :].transpose(1, 3, 2, 0)),
            "h0_in": h0_host,
            "c0_in": np.ascontiguousarray(c0_r[:, :, j, :].transpose(0, 2, 1)),
            "bias_in": np.ascontiguousarray(bb[:, :, j, :].transpose(2, 0, 1)),
            "wdec_in": wdec_host,
            "bdec_in": bdec_host,
        })

    nc = _get_nc(steps)
    res = run_bass_kernel_spmd(nc, in_maps, core_ids=list(range(NCORES)))

    r0 = res.results[0]
    logits = np.ascontiguousarray(
        r0["logits_out"].transpose(2, 0, 1))               # [B, steps, V]
    hT = np.ascontiguousarray(
        r0["hT_out"].transpose(0, 3, 2, 1)).reshape(L, B, H)
    cT = np.empty((L, B, H), np.float32)
    for j in range(NCORES):
        cj = res.results[j]["cT_out"]                      # [L, 128, B]
        cT[:, :, j * 128:(j + 1) * 128] = cj.transpose(0, 2, 1)
    return logits, hT, cT
